# revision 6
# baseline (speedup 1.0000x reference)
"""GCN layer kernel for 8 Trainium2 NeuronCores.

out = segment_sum(edge_vals * (features @ W)[edge_src], edge_dst) + bias
    = segment_sum(edge_vals * features[edge_src], edge_dst) @ W + bias

Strategy (dst-sharded, gather-based aggregation):
- Destination nodes sharded across 8 cores (12500/core, padded to 12544).
- Feature table split into 4 row-segments of 25000 so gather indices fit
  the int16 SWDGE dma_gather index format.
- Per (core, segment): dsts degree-sorted into their own slot permutation;
  windows of 128 dsts; a window's slot-k column gathers the k-th neighbor
  row for each of its 128 dsts. Greedy "pieces" of up to 8 windows /
  48 columns form one dma_gather each (batched descriptor generation —
  this removes the per-128-row indirect-DMA bottleneck of the previous
  version).
- DVE does one broadcast multiply (stride-0 val AP) per piece and a
  log2(K) pairwise tree-fold to produce the per-window aggregates, which
  dma_scatter_add accumulates into the output DRAM buffer in original
  dst order (combining the 4 per-segment permutations in DMA hardware).
- Final pass: read back aggregate tiles, PE transpose + matmul by W,
  add bias, write out.

The program is identical on all cores (SPMD): window column counts are
maxed across cores; padding uses idx=0/val=0.
"""
import os
import sys
from contextlib import ExitStack

import numpy as np

_REPO = "/opt/trn_rl_repo"
if _REPO not in sys.path:
    sys.path.insert(0, _REPO)

N_NODES = 100000
N_EDGES = 3200000
DIM = 128
N_CORES = 8
P = 128
N_SEG = 4
PIECE_COLS = 48
PIECE_WIN_MAX = 8


def _init_device():
    """Initialize the axon PJRT backend BEFORE bass/bacc machinery runs.

    If the backend first initializes after the bass build, the client
    comes up in a state where NTFF profile-start fails and the client
    dies. Initializing it first avoids that entirely.
    """
    try:
        import jax
        import jax.numpy as jnp

        jax.devices()
        jnp.zeros(8).block_until_ready()
    except Exception:
        pass


def _plan(edge_src, edge_dst, edge_vals, n_nodes, n_cores, n_seg):
    """Build the shared piece geometry + per-core streams.

    Returns (pieces, streams) where pieces is a list of dicts with
    seg/col0/ncols/nwin/nK/sc0 (shared across cores) and streams[c] has
    idx16 [128, total_cols*8] int16, val [128, total_cols] f32,
    sc16 [128, n_seg*shard_pad/16] int16.
    """
    shard = n_nodes // n_cores
    n_win = (shard + P - 1) // P
    shard_pad = n_win * P
    seg_rows = n_nodes // n_seg

    # per (core, seg): sorted edge arrays, degree, starts, slot->dst rows
    percs = [[None] * n_seg for _ in range(n_cores)]
    kw = np.zeros((n_cores, n_seg, n_win), dtype=np.int64)
    core_of = edge_dst // shard
    for c in range(n_cores):
        mc = core_of == c
        src_c = edge_src[mc]
        dst_c = edge_dst[mc] - c * shard
        val_c = edge_vals[mc]
        seg_of = src_c // seg_rows
        for s in range(n_seg):
            ms = seg_of == s
            ss = (src_c[ms] - s * seg_rows).astype(np.int64)
            ds = dst_c[ms]
            vs = val_c[ms]
            order = np.argsort(ds, kind="stable")
            ss, vs = ss[order], vs[order]
            deg = np.bincount(ds, minlength=shard)
            starts = np.concatenate([[0], np.cumsum(deg)[:-1]])
            perm = np.argsort(-deg, kind="stable")  # slot i -> dst perm[i]
            rows = np.concatenate(
                [perm, np.arange(shard, shard_pad)]
            )  # slot -> output row (bijection on [0, shard_pad))
            degp = np.concatenate([deg[perm], np.zeros(shard_pad - shard, np.int64)])
            percs[c][s] = {
                "src": ss,
                "val": vs,
                "deg": deg,
                "starts": starts,
                "rows": rows,
                "degp": degp,
            }
            kw[c, s] = np.maximum(
                degp.reshape(n_win, P).max(axis=1), 1
            )  # >=1 so every window exists

    kshared = kw.max(axis=0)  # [n_seg, n_win] shared across cores

    # greedy pieces per segment
    pieces = []
    col0 = 0
    sc0 = 0
    for s in range(n_seg):
        w = 0
        while w < n_win:
            nwin = 1
            kmax = int(kshared[s, w])
            while (
                w + nwin < n_win
                and nwin < PIECE_WIN_MAX
                and (nwin + 1) * max(kmax, int(kshared[s, w + nwin])) <= PIECE_COLS
            ):
                kmax = max(kmax, int(kshared[s, w + nwin]))
                nwin += 1
            ncols = nwin * kmax
            assert ncols <= PIECE_COLS, (s, w, nwin, kmax)
            pieces.append(
                {"seg": s, "w0": w, "col0": col0, "ncols": ncols, "nwin": nwin,
                 "nK": kmax, "sc0": sc0}
            )
            col0 += ncols
            sc0 += nwin * P
            w += nwin
    total_cols = col0
    total_sc = sc0  # == n_seg * shard_pad

    # per-core streams
    streams = []
    for c in range(n_cores):
        idxa = np.zeros((total_cols, P), np.int16)
        vala = np.zeros((total_cols, P), np.float32)
        sca = np.zeros(total_sc, np.int16)
        for pc in pieces:
            d = percs[c][pc["seg"]]
            w0, nwin, kmax, col0 = pc["w0"], pc["nwin"], pc["nK"], pc["col0"]
            slots = w0 * P + np.arange(nwin * P)
            rows = d["rows"][slots]
            sca[pc["sc0"]:pc["sc0"] + nwin * P] = rows
            degs = d["degp"][slots]
            for k in range(kmax):
                has = degs > k
                e = d["starts"][rows[has]] + k
                cols = col0 + (slots[has] - w0 * P) // P * kmax + k
                idxa[cols, slots[has] % P] = d["src"][e]
                vala[cols, slots[has] % P] = d["val"][e]
        # wrap idx stream (position i -> [i%16, i//16], replicated to 128)
        flat = idxa.reshape(-1)  # position j*128+p
        idx16 = np.tile(flat.reshape(-1, 16).T, (8, 1))
        sc16 = np.tile(sca.reshape(-1, 16).T, (8, 1))
        streams.append({"idx16": idx16, "val": vala.T.copy(), "sc16": sc16})
    geo = {
        "shard": shard,
        "n_win": n_win,
        "shard_pad": shard_pad,
        "seg_rows": seg_rows,
        "total_cols": total_cols,
        "total_sc": total_sc,
    }
    return pieces, streams, geo


def _build_nc(pieces, geo, n_cores):
    import concourse.bass as bass
    import concourse.tile as tile
    from concourse import bacc, mybir

    n_win = geo["n_win"]
    shard_pad = geo["shard_pad"]
    seg_rows = geo["seg_rows"]
    total_cols = geo["total_cols"]
    total_sc = geo["total_sc"]

    nc = bacc.Bacc(
        "TRN2", target_bir_lowering=False, debug=False, num_devices=n_cores
    )
    feat_ts = [
        nc.dram_tensor(f"feat{s}", [seg_rows, DIM], mybir.dt.float32,
                       kind="ExternalInput")
        for s in range(N_SEG)
    ]
    idx_t = nc.dram_tensor("idx16", [P, total_cols * P // 16], mybir.dt.int16,
                           kind="ExternalInput")
    val_t = nc.dram_tensor("vals", [P, total_cols], mybir.dt.float32,
                           kind="ExternalInput")
    sc_t = nc.dram_tensor("sc16", [P, total_sc // 16], mybir.dt.int16,
                          kind="ExternalInput")
    w_t = nc.dram_tensor("weight", [DIM, DIM], mybir.dt.float32,
                         kind="ExternalInput")
    bias_t = nc.dram_tensor("bias_tile", [P, DIM], mybir.dt.float32,
                            kind="ExternalInput")
    ident_t = nc.dram_tensor("identity", [P, P], mybir.dt.float32,
                             kind="ExternalInput")
    out_t = nc.dram_tensor("outp", [shard_pad, DIM], mybir.dt.float32,
                           kind="ExternalOutput")

    add_op = mybir.AluOpType.add
    mult_op = mybir.AluOpType.mult

    with tile.TileContext(nc) as tc:
        with ExitStack() as ctx:
            const = ctx.enter_context(tc.tile_pool(name="const", bufs=1))
            gpool = ctx.enter_context(tc.tile_pool(name="gather", bufs=3))
            apool = ctx.enter_context(tc.tile_pool(name="agg", bufs=3))
            fpool = ctx.enter_context(tc.tile_pool(name="fin", bufs=3))
            tpool = ctx.enter_context(tc.tile_pool(name="tr", bufs=3))
            opool = ctx.enter_context(tc.tile_pool(name="outw", bufs=3))
            pspool = ctx.enter_context(
                tc.tile_pool(name="psum", bufs=4, space="PSUM")
            )

            idx_all = const.tile([P, total_cols * P // 16], mybir.dt.int16)
            nc.sync.dma_start(idx_all[:], idx_t[:])
            sc_all = const.tile([P, total_sc // 16], mybir.dt.int16)
            nc.sync.dma_start(sc_all[:], sc_t[:])
            val_all = const.tile([P, total_cols], mybir.dt.float32)
            nc.sync.dma_start(val_all[:], val_t[:])
            w_tile = const.tile([DIM, DIM], mybir.dt.float32)
            nc.sync.dma_start(w_tile[:], w_t[:])
            bias_tile = const.tile([P, DIM], mybir.dt.float32)
            nc.sync.dma_start(bias_tile[:], bias_t[:])
            ident = const.tile([P, P], mybir.dt.float32)
            nc.sync.dma_start(ident[:], ident_t[:])
            zero8 = const.tile([P, 8, DIM], mybir.dt.float32)
            nc.vector.memset(zero8[:], 0.0)

            # zero the accumulator (outp)
            for b in range(0, n_win, 8):
                nb = min(8, n_win - b)
                view = out_t[b * P:(b + nb) * P].rearrange(
                    "(j p) f -> p j f", p=P
                )
                nc.sync.dma_start(view, zero8[:, :nb, :])

            # phase 1: gather / scale / fold / scatter-add
            for pc in pieces:
                s, col0, ncols, nwin, kmax = (
                    pc["seg"], pc["col0"], pc["ncols"], pc["nwin"], pc["nK"]
                )
                g = gpool.tile([P, PIECE_COLS, DIM], mybir.dt.float32)
                gc = g[:, :ncols, :]
                nc.gpsimd.dma_gather(
                    out_ap=gc,
                    in_ap=feat_ts[s][:],
                    idxs_ap=idx_all[:, col0 * 8:(col0 + ncols) * 8],
                    num_idxs=ncols * P,
                    num_idxs_reg=ncols * P,
                    elem_size=DIM,
                )
                vb = val_all[:, col0:col0 + ncols].unsqueeze(2).to_broadcast(
                    [P, ncols, DIM]
                )
                nc.vector.tensor_tensor(out=gc, in0=gc, in1=vb, op=mult_op)
                v = gc.rearrange("p (w k) f -> p w k f", w=nwin)
                agg = apool.tile([P, PIECE_WIN_MAX, DIM], mybir.dt.float32)
                h = kmax
                while h > 2:
                    m = h // 2
                    nc.vector.tensor_tensor(
                        out=v[:, :, 0:m, :], in0=v[:, :, 0:m, :],
                        in1=v[:, :, h - m:h, :], op=add_op,
                    )
                    h -= m
                if h == 2:
                    nc.vector.tensor_tensor(
                        out=agg[:, :nwin, :],
                        in0=v[:, :, 0, :], in1=v[:, :, 1, :], op=add_op,
                    )
                else:
                    nc.vector.tensor_copy(agg[:, :nwin, :], v[:, :, 0, :])
                nc.gpsimd.dma_scatter_add(
                    out_ap=out_t[:],
                    in_ap=agg[:, :nwin, :],
                    idxs_ap=sc_all[:, pc["sc0"] // 16:(pc["sc0"] + nwin * P) // 16],
                    num_idxs=nwin * P,
                    num_idxs_reg=nwin * P,
                    elem_size=DIM,
                )

            # phase 2: aggregate tiles -> @W + bias -> outp
            for b in range(0, n_win, 4):
                nb = min(4, n_win - b)
                view = out_t[b * P:(b + nb) * P].rearrange(
                    "(j p) f -> p j f", p=P
                )
                acc4 = fpool.tile([P, 4, DIM], mybir.dt.float32)
                nc.sync.dma_start(acc4[:, :nb, :], view)
                ow4 = opool.tile([P, 4, DIM], mybir.dt.float32)
                for j in range(nb):
                    ps_t = pspool.tile([P, P], mybir.dt.float32)
                    nc.tensor.transpose(
                        out=ps_t[:], in_=acc4[:, j, :], identity=ident[:]
                    )
                    tr = tpool.tile([P, P], mybir.dt.float32)
                    nc.vector.tensor_copy(tr[:], ps_t[:])
                    ps_o = pspool.tile([P, DIM], mybir.dt.float32)
                    nc.tensor.matmul(
                        out=ps_o[:], lhsT=tr[:], rhs=w_tile[:],
                        start=True, stop=True,
                    )
                    nc.vector.tensor_add(ow4[:, j, :], ps_o[:], bias_tile[:])
                nc.sync.dma_start(view, ow4[:, :nb, :])
    nc.compile()
    return nc


def kernel(features, edge_src, edge_dst, edge_vals, weight, bias):
    _init_device()
    features = np.ascontiguousarray(np.asarray(features), dtype=np.float32)
    edge_src = np.asarray(edge_src).astype(np.int64)
    edge_dst = np.asarray(edge_dst).astype(np.int64)
    edge_vals = np.asarray(edge_vals).astype(np.float32)
    weight = np.asarray(weight).astype(np.float32)
    bias = np.asarray(bias).astype(np.float32)

    pieces, streams, geo = _plan(
        edge_src, edge_dst, edge_vals, N_NODES, N_CORES, N_SEG
    )
    nc = _build_nc(pieces, geo, N_CORES)

    from concourse.bass_utils import run_bass_kernel_spmd

    seg_rows = geo["seg_rows"]
    bias_tile = np.tile(bias[None, :], (P, 1)).astype(np.float32)
    ident = np.eye(P, dtype=np.float32)
    in_maps = []
    for c in range(N_CORES):
        im = {
            f"feat{s}": np.ascontiguousarray(
                features[s * seg_rows:(s + 1) * seg_rows]
            )
            for s in range(N_SEG)
        }
        im.update(
            {
                "idx16": streams[c]["idx16"],
                "vals": streams[c]["val"],
                "sc16": streams[c]["sc16"],
                "weight": weight,
                "bias_tile": bias_tile,
                "identity": ident,
            }
        )
        in_maps.append(im)
    trace = os.environ.get("GCN_TRACE", "0") == "1"

    res = None
    for attempt in range(4):
        try:
            res = run_bass_kernel_spmd(
                nc, in_maps, core_ids=list(range(N_CORES)), trace=trace
            )
            break
        except Exception:
            if attempt == 3:
                raise
            import time as _time

            _time.sleep(10.0)

    if trace:
        print(f"HW exec time: {res.exec_time_ns} ns")
        kernel.last_exec_time_ns = res.exec_time_ns

    shard = geo["shard"]
    out = np.empty((N_NODES, DIM), dtype=np.float32)
    for c in range(N_CORES):
        out[c * shard:(c + 1) * shard] = res.results[c]["outp"][:shard]
    return out


kernel.last_exec_time_ns = None
